# revision 10
# baseline (speedup 1.0000x reference)
"""Trainium2 Bass kernel for nn_CNN_CDR123_global_max (8-core data parallel).

Architecture (per core, batch shard of 2048 rows):
  - Conv+ReLU+global-max for each of 7 sequences is expressed as matmuls
    against a host-built block-Toeplitz "im2col on weights" matrix
    G_i [20*L, 80*L] (80 = 5 kernel sizes x 16 filters), columns laid out
    position-major (lo, f).  The batch tile (128 rows) is the stationary
    operand (x transposed on host to [20L, B]); G is the moving operand.
    PSUM accumulates over K-chunks of 128.
  - DVE reduce_max over positions (strided AP view), ACT relu -> feats bf16.
  - feats [128b, 560] DMA-transposed (XBAR, bf16) to [560, B] chunks.
  - lin1 (560->64) + sigmoid, lin2 (64->1) on PE/ACT.  Output [1, 2048] f32.
"""

import numpy as np
import ml_dtypes

SEQS = [("pep", 12), ("a1", 7), ("a2", 8), ("a3", 16), ("b1", 6), ("b2", 7), ("b3", 18)]
KSIZES = (1, 3, 5, 7, 9)
C = 20
F = 16
FA = len(KSIZES) * F  # 80
NCORES = 8
BATCH = 16384
BC = BATCH // NCORES  # 2048
NBT = BC // 128  # 16 batch tiles per core
LIN_IN = 560
LIN_DIM = 64

BF16 = ml_dtypes.bfloat16

_CACHE = {}


def _seq_meta():
    metas = []
    for name, L in SEQS:
        KP = C * L
        nK = (KP + 127) // 128
        KPp = nK * 128
        cols = FA * L
        # uniform 6-position chunks of 480 cols, plus a tail chunk
        nN0 = L // 6
        tail = L - 6 * nN0
        nN = nN0 + (1 if tail else 0)
        metas.append(dict(name=name, L=L, KP=KP, nK=nK, KPp=KPp, cols=cols,
                          nN0=nN0, tail=tail, nN=nN))
    return metas


def _build_g(name, L, Wk_by_k):
    """G [20L, 80L] f32: G[(c*L+li), (lo*80 + ko*16 + f)] = W_k[f, c, li-lo+pad]/5."""
    G = np.zeros((C * L, L * FA), dtype=np.float32)
    for ko, k in enumerate(KSIZES):
        W = Wk_by_k[k]  # [F, C, k]
        pad = (k - 1) // 2
        for lo in range(L):
            for t in range(k):
                li = lo + t - pad
                if 0 <= li < L:
                    # rows c*L+li for all c; cols lo*80 + ko*16 + f
                    G[li::L, lo * FA + ko * F:lo * FA + ko * F + F] += \
                        W[:, :, t].T / 5.0
    return G


def _build_nc():
    import concourse.bass as bass
    import concourse.tile as tile
    import concourse.mybir as mybir
    from concourse import bass_utils
    from concourse.vector_clock import ScopedClock

    # -- workaround: this walrus build accepts only one sync-wait per CTRL
    #    instruction; split the Tile tail-drain waits across SP nops. --
    bass_utils.upload_artifacts = lambda tmpdir: "local://" + tmpdir

    def _split_drain_and_barrier(self, tick_clock, wait_clock):
        nc = self.nc
        drain_inst = nc.sync.drain()
        wait_clock.add_sem_waits(
            drain_inst.ins, ScopedClock({None: tick_clock.global_clock}))
        ins = drain_inst.ins
        si = ins.sync_info
        if si is not None and si.on_wait and len(si.on_wait) > 1:
            waits = list(si.on_wait)
            bb = nc.cur_bb.bb
            assert bb.instructions[-1] is ins
            bb.instructions.pop()
            for w in waits:
                nop = nc.sync.nop(hint="drain_wait_split", nofuse=True)
                nsi = nop.ins.sync_info
                if nsi is None:
                    nop.ins.sync_info = mybir.SyncInfo(on_wait=[w], on_update=[])
                else:
                    nsi.on_wait = [w]
            si.on_wait = []
            bb.instructions.append(ins)
        nc.all_engine_barrier()
        popped = nc._tile_sem_poison_stack.pop()
        assert popped is self._sem_poison
        nc.clear_and_free_semaphores(list(self.sems.allocated().values()))
        nc.all_engine_barrier()

    tile.TileContext._drain_and_barrier = _split_drain_and_barrier

    def _split_waits(nc, cap=1):
        """Walrus in this build accepts very few sync-waits per instruction.
        Hoist all but `cap` waits of every instruction onto same-engine
        nops inserted immediately before it (same AND semantics, since the
        sequencer processes waits in stream order)."""
        ctr = [0]
        for f in nc.m.functions:
            for blk in f.blocks:
                new = []
                for inst in blk.instructions:
                    si = inst.sync_info
                    if si is not None and si.on_wait and len(si.on_wait) > cap:
                        waits = list(si.on_wait)
                        for w in waits[:-cap] if cap else waits:
                            ctr[0] += 1
                            nop = mybir.InstNoOp(
                                name=f"waitsplit-{ctr[0]}", ins=[], outs=[])
                            nop.engine = inst.engine
                            nop.sync_info = mybir.SyncInfo(
                                on_wait=[w], on_update=[])
                            nop.bass_nofuse = True
                            new.append(nop)
                        si.on_wait = waits[-cap:] if cap else []
                    new.append(inst)
                blk.instructions[:] = new

    metas = _seq_meta()
    dt = mybir.dt
    nc = bass.Bass()

    xds, gds = [], []
    for m in metas:
        xds.append(nc.declare_dram_parameter(
            f"x_{m['name']}", [m["KPp"], BC], dt.bfloat16, isOutput=False))
        gds.append(nc.declare_dram_parameter(
            f"g_{m['name']}", [m["nK"], 128, m["cols"]], dt.bfloat16, isOutput=False))
    w1d = nc.declare_dram_parameter("w1t", [5, 128, LIN_DIM], dt.bfloat16, isOutput=False)
    b1d = nc.declare_dram_parameter("b1", [LIN_DIM, 1], dt.float32, isOutput=False)
    w2d = nc.declare_dram_parameter("w2", [LIN_DIM, 1], dt.float32, isOutput=False)
    b2d = nc.declare_dram_parameter("b2", [1, 1], dt.float32, isOutput=False)
    outd = nc.declare_dram_parameter("out", [1, BC], dt.float32, isOutput=True)

    with tile.TileContext(nc) as tc:
        with (
            tc.tile_pool(name="gpool", bufs=1) as gpool,
            tc.tile_pool(name="xpool", bufs=6) as xpool,
            tc.tile_pool(name="redpool", bufs=4) as redpool,
            tc.tile_pool(name="featpool", bufs=1) as featpool,
            tc.tile_pool(name="ftpool", bufs=1) as ftpool,
            tc.tile_pool(name="linpool", bufs=1) as linpool,
        ):
            # persistent tiles
            feats = featpool.tile([128, NBT, 640], dt.bfloat16)  # (b, bt, col)
            nc.gpsimd.memset(feats[:, :, LIN_IN:640], 0.0)
            featsT = ftpool.tile([128, 5, BC], dt.bfloat16)          # (colchunk, bt*b)
            w1_sb = linpool.tile([128, 5, LIN_DIM], dt.bfloat16)
            b1_sb = linpool.tile([LIN_DIM, 1], dt.float32)
            w2_sb = linpool.tile([LIN_DIM, 1], dt.float32)
            b2_sb = linpool.tile([1, 1], dt.float32)
            out_sb = linpool.tile([1, BC], dt.float32)

            nc.sync.dma_start(w1_sb[:], w1d[:].rearrange("j p c -> p j c"))
            nc.sync.dma_start(b1_sb[:], b1d[:])
            nc.sync.dma_start(w2_sb[:], w2d[:])
            nc.sync.dma_start(b2_sb[:], b2d[:])

            col_off = 0
            with tc.tile_pool(name="pspool", bufs=2, space="PSUM") as pspool:
              for si, m in enumerate(metas):
                L, nK, nN0, tail = m["L"], m["nK"], m["nN0"], m["tail"]
                nN = m["nN"]
                g_sb = gpool.tile([128, nK, m["cols"]], dt.bfloat16,
                                  tag=f"g{si}")
                nc.sync.dma_start(g_sb[:], gds[si][:].rearrange("k p c -> p k c"))
                xts = []
                for k in range(nK):
                    xt = xpool.tile([128, BC], dt.bfloat16, tag="x")
                    nc.sync.dma_start(xt[:], xds[si][k * 128:(k + 1) * 128, :])
                    xts.append(xt)

                for bt in range(NBT):
                    bsl = slice(bt * 128, (bt + 1) * 128)
                    ps = pspool.tile([128, nN, 512], dt.float32, tag="ps")
                    for k in range(nK):
                        lhsT = xts[k][:, bsl]
                        for n in range(nN0):
                            nc.tensor.matmul(
                                ps[:, n, 0:480], lhsT,
                                g_sb[:, k, n * 480:(n + 1) * 480],
                                start=(k == 0), stop=(k == nK - 1))
                        if tail:
                            nc.tensor.matmul(
                                ps[:, nN0, 0:tail * FA], lhsT,
                                g_sb[:, k, nN0 * 480:nN0 * 480 + tail * FA],
                                start=(k == 0), stop=(k == nK - 1))
                    # max over positions: one strided reduce for the uniform
                    # chunks, one for the tail, combine, then relu on ACT.
                    r0 = redpool.tile([128, FA], dt.float32, tag="r0")
                    uni = ps[:, 0:nN0, 0:480].rearrange(
                        "p n (l f) -> p f n l", f=FA)
                    nc.vector.reduce_max(r0[:], uni, axis=mybir.AxisListType.XY)
                    if tail:
                        r1 = redpool.tile([128, FA], dt.float32, tag="r1")
                        tl = ps[:, nN0, 0:tail * FA].rearrange(
                            "p (l f) -> p f l", f=FA)
                        nc.vector.reduce_max(r1[:], tl, axis=mybir.AxisListType.X)
                        nc.vector.tensor_max(r0[:], r0[:], r1[:])
                    nc.scalar.activation(
                        feats[:, bt, col_off:col_off + FA], r0[:],
                        mybir.ActivationFunctionType.Relu)
                col_off += FA

            # transpose feats -> featsT (XBAR DMA, bf16, 128x128 tiles)
            for bt in range(NBT):
                for j in range(5):
                    nc.sync.dma_start(
                        featsT[:, j, bt * 128:(bt + 1) * 128],
                        feats[:, bt, j * 128:(j + 1) * 128],
                        transpose=True)

            # lin1 + sigmoid + lin2
            with tc.tile_pool(name="lpsum", bufs=2, space="PSUM") as lpsum:
              for nb in range(4):
                csl = slice(nb * 512, (nb + 1) * 512)
                ph = lpsum.tile([LIN_DIM, 512], dt.float32, tag="ph")
                for j in range(5):
                    nc.tensor.matmul(ph[:], w1_sb[:, j, :], featsT[:, j, csl],
                                     start=(j == 0), stop=(j == 4))
                h_sb = linpool.tile([LIN_DIM, 512], dt.float32, tag="h")
                nc.scalar.activation(h_sb[:], ph[:],
                                     mybir.ActivationFunctionType.Sigmoid,
                                     bias=b1_sb[:, 0:1])
                po = lpsum.tile([1, 512], dt.float32, tag="po")
                nc.tensor.matmul(po[:], w2_sb[:], h_sb[:])
                nc.scalar.activation(out_sb[:, csl], po[:],
                                     mybir.ActivationFunctionType.Identity,
                                     bias=b2_sb[:, 0:1])

            nc.sync.dma_start(outd[:], out_sb[:])

    _split_waits(nc, cap=1)
    return nc


def _get_nc():
    if "nc" not in _CACHE:
        _CACHE["nc"] = _build_nc()
    return _CACHE["nc"]


def _prep_inputs(inputs):
    """Host-side marshaling -> per-core in_maps."""
    metas = _seq_meta()
    Wk_by_k = {k: inputs[f"Wk{k}"] for k in KSIZES}  # [7, F, C, k]

    shared = {}
    for si, m in enumerate(metas):
        G = _build_g(m["name"], m["L"],
                     {k: np.asarray(Wk_by_k[k][si]) for k in KSIZES})
        Gp = np.zeros((m["KPp"], m["cols"]), dtype=np.float32)
        Gp[:m["KP"]] = G
        shared[f"g_{m['name']}"] = np.ascontiguousarray(
            Gp.reshape(m["nK"], 128, m["cols"]).astype(BF16))

    w1 = np.asarray(inputs["lin1_w"], dtype=np.float32)  # [64, 560]
    w1t = np.zeros((640, LIN_DIM), dtype=np.float32)
    w1t[:LIN_IN] = w1.T
    shared["w1t"] = np.ascontiguousarray(
        w1t.reshape(5, 128, LIN_DIM).astype(BF16))
    shared["b1"] = np.asarray(inputs["lin1_b"], dtype=np.float32).reshape(LIN_DIM, 1)
    shared["w2"] = np.ascontiguousarray(
        np.asarray(inputs["lin2_w"], dtype=np.float32).reshape(1, LIN_DIM).T)
    shared["b2"] = np.asarray(inputs["lin2_b"], dtype=np.float32).reshape(1, 1)

    in_maps = []
    percore_x = {}
    for m in metas:
        x = np.asarray(inputs[m["name"]], dtype=np.float32)  # [B, C, L]
        xt = np.zeros((m["KPp"], BATCH), dtype=np.float32)
        xt[:m["KP"]] = x.reshape(BATCH, m["KP"]).T  # row r = c*L + l
        percore_x[m["name"]] = xt.astype(BF16)
    for c in range(NCORES):
        im = dict(shared)
        csl = slice(c * BC, (c + 1) * BC)
        for m in metas:
            im[f"x_{m['name']}"] = np.ascontiguousarray(
                percore_x[m["name"]][:, csl])
        in_maps.append(im)
    return in_maps


def kernel(**inputs):
    from concourse import bass_utils
    nc = _get_nc()
    in_maps = _prep_inputs(inputs)
    res = bass_utils.run_bass_kernel_spmd(nc, in_maps, core_ids=list(range(NCORES)))
    out = np.concatenate([res.results[c]["out"][0] for c in range(NCORES)])
    return out.astype(np.float32)


def run_profiled(inputs, tmpdir=None):
    """test.py helper: returns (output, BassKernelResults with exec_time_ns)."""
    from concourse import bass_utils
    nc = _get_nc()
    in_maps = _prep_inputs(inputs)
    res = bass_utils.run_bass_kernel_spmd(
        nc, in_maps, core_ids=list(range(NCORES)), trace=True, tmpdir=tmpdir)
    out = np.concatenate([res.results[c]["out"][0] for c in range(NCORES)])
    return out.astype(np.float32), res


# revision 12
# speedup vs baseline: 1.1463x; 1.1463x over previous
"""Trainium2 Bass kernel for nn_CNN_CDR123_global_max (8-core data parallel).

Architecture (per core, batch shard of 2048 rows):
  - Conv+ReLU+global-max for each of 7 sequences is expressed as matmuls
    against a host-built block-Toeplitz "im2col on weights" matrix
    G_i [20*L, 80*L] (80 = 5 kernel sizes x 16 filters), columns laid out
    position-major (lo, f) and zero-padded to uniform 480-column chunks.
    The batch tile (128 rows) is the stationary operand (x transposed on
    host to [20L, B]); G is the moving operand.  PSUM accumulates over
    K-chunks of 128.
  - Global max over positions, split across engines:
      path A (DVE): single strided reduce_max per (seq, batch-tile)
      path B (ACT relu-cast to SBUF bf16 + DVE pairwise-max tree at 2x)
    Zero-padded position groups make max(...,0) == relu for padded seqs;
    unpadded seqs get an explicit ACT relu.
  - feats bf16 staged to DRAM, then 10 large XBAR DMA-transposes ->
    featsT [560, B] chunks.
  - lin1 (560->64, bf16) + sigmoid (f32), lin2 (64->1, f32).  Out [1,2048].
"""

import numpy as np
import ml_dtypes

SEQS = [("pep", 12), ("a1", 7), ("a2", 8), ("a3", 16), ("b1", 6), ("b2", 7), ("b3", 18)]
KSIZES = (1, 3, 5, 7, 9)
C = 20
F = 16
FA = len(KSIZES) * F  # 80
NCORES = 8
BATCH = 16384
BC = BATCH // NCORES  # 2048
NBT = BC // 128  # 16 batch tiles per core
LIN_IN = 560
LIN_DIM = 64

# seqs whose PSUM chunks are consumed by ACT relu-cast + DVE tree
PATH_B = {"a2", "a3", "b3"}
# seqs with no zero-padded position groups -> need explicit relu
NEED_RELU = {"pep", "b1"}

BF16 = ml_dtypes.bfloat16

_CACHE = {}


# per-seq uniform chunk size (positions per PSUM chunk), chosen to minimize
# zero-padding: cols = ceil(L/cs)*cs*80, chunk width cs*80 <= 512
CHUNK = {"pep": 6, "a1": 4, "a2": 4, "a3": 4, "b1": 6, "b2": 4, "b3": 6}


def _seq_meta():
    metas = []
    for name, L in SEQS:
        KP = C * L
        nK = (KP + 127) // 128
        KPp = nK * 128
        cs = CHUNK[name]
        nN = (L + cs - 1) // cs
        cols = nN * cs * FA          # zero-padded G columns
        metas.append(dict(name=name, L=L, KP=KP, nK=nK, KPp=KPp, cols=cols,
                          nN=nN, cs=cs, cw=cs * FA))
    return metas


def _build_g(name, L, Wk_by_k):
    """G [20L, nN*480] f32 (zero-padded cols):
    G[(c*L+li), (lo*80 + ko*16 + f)] = W_k[f, c, li-lo+pad]/5."""
    cs = CHUNK[name]
    nN = (L + cs - 1) // cs
    G = np.zeros((C * L, nN * cs * FA), dtype=np.float32)
    for ko, k in enumerate(KSIZES):
        W = Wk_by_k[k]  # [F, C, k]
        pad = (k - 1) // 2
        for lo in range(L):
            for t in range(k):
                li = lo + t - pad
                if 0 <= li < L:
                    G[li::L, lo * FA + ko * F:lo * FA + ko * F + F] += \
                        W[:, :, t].T / 5.0
    return G


def _build_nc():
    import concourse.bass as bass
    import concourse.tile as tile
    import concourse.mybir as mybir
    from concourse import bass_utils
    from concourse.vector_clock import ScopedClock

    bass_utils.upload_artifacts = lambda tmpdir: "local://" + tmpdir

    # -- workaround: this walrus build accepts only one sync-wait per
    #    instruction; split extra waits onto same-engine nops. --
    def _split_drain_and_barrier(self, tick_clock, wait_clock):
        nc = self.nc
        drain_inst = nc.sync.drain()
        wait_clock.add_sem_waits(
            drain_inst.ins, ScopedClock({None: tick_clock.global_clock}))
        ins = drain_inst.ins
        si = ins.sync_info
        if si is not None and si.on_wait and len(si.on_wait) > 1:
            waits = list(si.on_wait)
            bb = nc.cur_bb.bb
            assert bb.instructions[-1] is ins
            bb.instructions.pop()
            for w in waits:
                nop = nc.sync.nop(hint="drain_wait_split", nofuse=True)
                nsi = nop.ins.sync_info
                if nsi is None:
                    nop.ins.sync_info = mybir.SyncInfo(on_wait=[w], on_update=[])
                else:
                    nsi.on_wait = [w]
            si.on_wait = []
            bb.instructions.append(ins)
        nc.all_engine_barrier()
        popped = nc._tile_sem_poison_stack.pop()
        assert popped is self._sem_poison
        nc.clear_and_free_semaphores(list(self.sems.allocated().values()))
        nc.all_engine_barrier()

    tile.TileContext._drain_and_barrier = _split_drain_and_barrier

    def _split_waits(nc, cap=1):
        ctr = [0]
        for f in nc.m.functions:
            for blk in f.blocks:
                new = []
                for inst in blk.instructions:
                    si = inst.sync_info
                    if si is not None and si.on_wait and len(si.on_wait) > cap:
                        waits = list(si.on_wait)
                        for w in waits[:-cap] if cap else waits:
                            ctr[0] += 1
                            nop = mybir.InstNoOp(
                                name=f"waitsplit-{ctr[0]}", ins=[], outs=[])
                            nop.engine = inst.engine
                            nop.sync_info = mybir.SyncInfo(
                                on_wait=[w], on_update=[])
                            nop.bass_nofuse = True
                            new.append(nop)
                        si.on_wait = waits[-cap:] if cap else []
                    new.append(inst)
                blk.instructions[:] = new

    metas = _seq_meta()
    dt = mybir.dt
    nc = bass.Bass()

    xds, gds = [], []
    for m in metas:
        xds.append(nc.declare_dram_parameter(
            f"x_{m['name']}", [m["KPp"], BC], dt.bfloat16, isOutput=False))
        gds.append(nc.declare_dram_parameter(
            f"g_{m['name']}", [m["nK"], 128, m["cols"]], dt.bfloat16, isOutput=False))
    w1d = nc.declare_dram_parameter("w1t", [5, 128, LIN_DIM], dt.bfloat16, isOutput=False)
    b1d = nc.declare_dram_parameter("b1", [LIN_DIM, 1], dt.float32, isOutput=False)
    w2d = nc.declare_dram_parameter("w2", [LIN_DIM, 1], dt.float32, isOutput=False)
    b2d = nc.declare_dram_parameter("b2", [1, 1], dt.float32, isOutput=False)
    outd = nc.declare_dram_parameter("out", [1, BC], dt.float32, isOutput=True)
    feats_dram = nc.dram_tensor("feats_stage", [BC, 640], dt.bfloat16)

    with tile.TileContext(nc) as tc:
        with (
            tc.tile_pool(name="gpool", bufs=1) as gpool,
            tc.tile_pool(name="xpool", bufs=6) as xpool,
            tc.tile_pool(name="redpool", bufs=4) as redpool,
            tc.tile_pool(name="stpool", bufs=3) as stpool,
            tc.tile_pool(name="featpool", bufs=1) as featpool,
            tc.tile_pool(name="ftpool", bufs=1) as ftpool,
            tc.tile_pool(name="linpool", bufs=1) as linpool,
        ):
            # persistent tiles
            feats = featpool.tile([128, NBT, 640], dt.bfloat16)  # (b, bt, col)
            nc.gpsimd.memset(feats[:, :, LIN_IN:640], 0.0)
            featsT = ftpool.tile([128, 5, BC], dt.bfloat16)
            w1_sb = linpool.tile([128, 5, LIN_DIM], dt.bfloat16)
            b1_sb = linpool.tile([LIN_DIM, 1], dt.float32)
            w2_sb = linpool.tile([LIN_DIM, 1], dt.float32)
            b2_sb = linpool.tile([1, 1], dt.float32)
            out_sb = linpool.tile([1, BC], dt.float32)

            nc.sync.dma_start(w1_sb[:], w1d[:].rearrange("j p c -> p j c"))
            nc.sync.dma_start(b1_sb[:], b1d[:])
            nc.sync.dma_start(w2_sb[:], w2d[:])
            nc.sync.dma_start(b2_sb[:], b2d[:])

            col_off = 0
            with tc.tile_pool(name="pspool", bufs=2, space="PSUM") as pspool:
              for si, m in enumerate(metas):
                name, L, nK, nN = m["name"], m["L"], m["nK"], m["nN"]
                cs, cw = m["cs"], m["cw"]
                g_sb = gpool.tile([128, nK, m["cols"]], dt.bfloat16,
                                  tag=f"g{si}")
                nc.gpsimd.dma_start(g_sb[:], gds[si][:].rearrange("k p c -> p k c"))
                xts = []
                for k in range(nK):
                    xt = xpool.tile([128, BC], dt.bfloat16, tag="x")
                    nc.gpsimd.dma_start(xt[:], xds[si][k * 128:(k + 1) * 128, :])
                    xts.append(xt)

                for bt in range(NBT):
                    bsl = slice(bt * 128, (bt + 1) * 128)
                    ps = pspool.tile([128, nN, 512], dt.float32, tag="ps")
                    for k in range(nK):
                        lhsT = xts[k][:, bsl]
                        for n in range(nN):
                            nc.tensor.matmul(
                                ps[:, n, 0:cw], lhsT,
                                g_sb[:, k, n * cw:(n + 1) * cw],
                                start=(k == 0), stop=(k == nK - 1))
                    fsl = feats[:, bt, col_off:col_off + FA]
                    if name in PATH_B:
                        # ACT relu-cast PSUM -> SBUF bf16, DVE pairwise tree
                        st = stpool.tile([128, nN * cw], dt.bfloat16, tag="st")
                        nc.scalar.activation(
                            st[:].rearrange("p (n c) -> p n c", c=cw),
                            ps[:, :, 0:cw],
                            mybir.ActivationFunctionType.Relu)
                        g = cs * nN  # 80-col position groups
                        while g > 1:
                            h = g // 2
                            dst = st[:, 0:h * FA]
                            a = st[:, 0:h * FA]
                            b = st[:, (g - h) * FA:g * FA]
                            if g == 2:
                                nc.vector.tensor_max(fsl, a, b)
                            else:
                                nc.vector.tensor_max(dst, a, b)
                            g -= h
                    else:
                        # single strided reduce (includes zero-pad groups,
                        # so max>=0 == relu for padded seqs)
                        view = ps[:, :, 0:cw].rearrange(
                            "p n (l f) -> p f n l", f=FA)
                        if name in NEED_RELU:
                            r0 = redpool.tile([128, FA], dt.float32, tag="r0")
                            nc.vector.reduce_max(
                                r0[:], view, axis=mybir.AxisListType.XY)
                            nc.scalar.activation(
                                fsl, r0[:], mybir.ActivationFunctionType.Relu)
                        else:
                            nc.vector.reduce_max(
                                fsl, view, axis=mybir.AxisListType.XY)
                col_off += FA

            # feats -> DRAM, then 10 big XBAR transposes -> featsT
            for bt in range(NBT):
                nc.sync.dma_start(
                    feats_dram[bt * 128:(bt + 1) * 128, :], feats[:, bt, :])
            for j in range(5):
                for h in range(2):
                    nc.sync.dma_start(
                        featsT[:, j, h * 1024:(h + 1) * 1024],
                        feats_dram[h * 1024:(h + 1) * 1024,
                                   j * 128:(j + 1) * 128],
                        transpose=True)

            # lin1 + sigmoid + lin2
            with tc.tile_pool(name="lpsum", bufs=2, space="PSUM") as lpsum:
              for nb in range(4):
                csl = slice(nb * 512, (nb + 1) * 512)
                ph = lpsum.tile([LIN_DIM, 512], dt.float32, tag="ph")
                for j in range(5):
                    nc.tensor.matmul(ph[:], w1_sb[:, j, :], featsT[:, j, csl],
                                     start=(j == 0), stop=(j == 4))
                h_sb = linpool.tile([LIN_DIM, 512], dt.float32, tag="h")
                nc.scalar.activation(h_sb[:], ph[:],
                                     mybir.ActivationFunctionType.Sigmoid,
                                     bias=b1_sb[:, 0:1])
                po = lpsum.tile([1, 512], dt.float32, tag="po")
                nc.tensor.matmul(po[:], w2_sb[:], h_sb[:])
                nc.scalar.activation(out_sb[:, csl], po[:],
                                     mybir.ActivationFunctionType.Identity,
                                     bias=b2_sb[:, 0:1])

            nc.sync.dma_start(outd[:], out_sb[:])

    _split_waits(nc, cap=1)
    return nc


def _get_nc():
    if "nc" not in _CACHE:
        _CACHE["nc"] = _build_nc()
    return _CACHE["nc"]


def _prep_inputs(inputs):
    """Host-side marshaling -> per-core in_maps."""
    metas = _seq_meta()
    Wk_by_k = {k: inputs[f"Wk{k}"] for k in KSIZES}  # [7, F, C, k]

    shared = {}
    for si, m in enumerate(metas):
        G = _build_g(m["name"], m["L"],
                     {k: np.asarray(Wk_by_k[k][si]) for k in KSIZES})
        Gp = np.zeros((m["KPp"], m["cols"]), dtype=np.float32)
        Gp[:m["KP"]] = G
        shared[f"g_{m['name']}"] = np.ascontiguousarray(
            Gp.reshape(m["nK"], 128, m["cols"]).astype(BF16))

    w1 = np.asarray(inputs["lin1_w"], dtype=np.float32)  # [64, 560]
    w1t = np.zeros((640, LIN_DIM), dtype=np.float32)
    w1t[:LIN_IN] = w1.T
    shared["w1t"] = np.ascontiguousarray(
        w1t.reshape(5, 128, LIN_DIM).astype(BF16))
    shared["b1"] = np.asarray(inputs["lin1_b"], dtype=np.float32).reshape(LIN_DIM, 1)
    shared["w2"] = np.ascontiguousarray(
        np.asarray(inputs["lin2_w"], dtype=np.float32).reshape(1, LIN_DIM).T)
    shared["b2"] = np.asarray(inputs["lin2_b"], dtype=np.float32).reshape(1, 1)

    in_maps = []
    percore_x = {}
    for m in metas:
        x = np.asarray(inputs[m["name"]], dtype=np.float32)  # [B, C, L]
        xt = np.zeros((m["KPp"], BATCH), dtype=np.float32)
        xt[:m["KP"]] = x.reshape(BATCH, m["KP"]).T  # row r = c*L + l
        percore_x[m["name"]] = xt.astype(BF16)
    for c in range(NCORES):
        im = dict(shared)
        csl = slice(c * BC, (c + 1) * BC)
        for m in metas:
            im[f"x_{m['name']}"] = np.ascontiguousarray(
                percore_x[m["name"]][:, csl])
        in_maps.append(im)
    return in_maps


def kernel(**inputs):
    from concourse import bass_utils
    nc = _get_nc()
    in_maps = _prep_inputs(inputs)
    res = bass_utils.run_bass_kernel_spmd(nc, in_maps, core_ids=list(range(NCORES)))
    out = np.concatenate([res.results[c]["out"][0] for c in range(NCORES)])
    return out.astype(np.float32)


def run_profiled(inputs, tmpdir=None):
    """test.py helper: returns (output, BassKernelResults with exec_time_ns)."""
    from concourse import bass_utils
    nc = _get_nc()
    in_maps = _prep_inputs(inputs)
    res = bass_utils.run_bass_kernel_spmd(
        nc, in_maps, core_ids=list(range(NCORES)), trace=True, tmpdir=tmpdir)
    out = np.concatenate([res.results[c]["out"][0] for c in range(NCORES)])
    return out.astype(np.float32), res
